# revision 15
# baseline (speedup 1.0000x reference)
"""Trainium2 Bass kernel for nn_DistanceEncoder (gnn_message_passing).

Reference math (per batch b of 2, n=512 nodes, hid=128):
  dist = cdist(x, x)                               (n, n)
  h    = MLP0(dist[..., None]); h = MLP1(h); h = MLP2(h)   per-edge (n, n, 128)
  out  = mean_j(h) @ Wo + bo                       (n, 128)

Host-side algebraic folding (exact up to fp rounding):
  a0 = SiLU(d * w1_0 + b1_0)
  t1 = a0 @ A + c1,  A = W2_0 @ W1_1,  c1 = b2_0 @ W1_1 + b1_1
  a1 = SiLU(t1)
  t2 = a1 @ Bm + c2, Bm = W2_1 @ W1_2, c2 = b2_1 @ W1_2 + b1_2
  a2 = SiLU(t2)
  S_i = sum_j a2_ij
  out_i = S_i @ Cs + c3,  Cs = (W2_2 @ Wo)/512,  c3 = b2_2 @ Wo + bo

Sharding: the 2*512=1024 query rows are split across 8 cores (128 each; cores
0-3 handle batch 0, cores 4-7 batch 1). Each core sees all 512 sources of its
batch; the mean-aggregation is local, no collectives.

Distances are computed on the PE from augmented features:
  d2[i, j] = x_i . x_i + x_j . x_j - 2 x_i . x_j  (K=4 matmul), then relu+sqrt.
"""

import os
from contextlib import ExitStack

import numpy as np
import ml_dtypes

import concourse.bacc as bacc
import concourse.bass as bass
import concourse.mybir as mybir
import concourse.tile as tile
from concourse.bass_utils import run_bass_kernel_spmd

N_CORES = 8
B, N, HID = 2, 512, 128
QPC = (B * N) // N_CORES  # 128 queries per core
F32 = mybir.dt.float32
AF = mybir.ActivationFunctionType

# matmul precision mode: "f32r" (fp32 data, reduced-precision PE pass at full
# rate), "bf16", or "f32" (4x slower PE)
MM_MODE = os.environ.get("DE_MM_MODE", "bf16")
# "sym": symmetric pair sharding (each edge computed once globally, partial
# sums combined during the host gather); "basic": plain query sharding.
ALGO = os.environ.get("DE_ALGO", "sym")
# activation used on device; Silu is the real one. "sigmoid" only for CoreSim
# debugging (the python interp lacks Silu).
ACT_NAME = os.environ.get("DE_ACT", "silu")
QB = 2  # queries batched per activation instruction


def _act_fn():
    return AF.Silu if ACT_NAME == "silu" else AF.Sigmoid


def _mm_tile_dt():
    if MM_MODE == "bf16":
        return mybir.dt.bfloat16
    if MM_MODE == "f32r":
        return mybir.dt.float32r
    return F32


def _mm_ap(ap):
    return ap


def build_nc():
    nc = bacc.Bacc("TRN2", target_bir_lowering=False)
    mdt = _mm_tile_dt()

    # DRAM I/O (per-core shapes)
    d_xq = nc.dram_tensor("xq", [4, QPC], F32, kind="ExternalInput")
    d_xs = nc.dram_tensor("xs", [4, N], F32, kind="ExternalInput")
    d_w10 = nc.dram_tensor("w10", [1, HID], mdt, kind="ExternalInput")
    d_A = nc.dram_tensor("A", [HID, HID], mdt, kind="ExternalInput")
    d_Bm = nc.dram_tensor("Bm", [HID, HID], mdt, kind="ExternalInput")
    d_Cs = nc.dram_tensor("Cs", [HID, HID], F32, kind="ExternalInput")
    d_b10 = nc.dram_tensor("b10", [HID, 1], F32, kind="ExternalInput")
    d_c1 = nc.dram_tensor("c1", [HID, 1], F32, kind="ExternalInput")
    d_c2 = nc.dram_tensor("c2", [HID, 1], F32, kind="ExternalInput")
    d_c3 = nc.dram_tensor("c3", [HID, 1], F32, kind="ExternalInput")
    d_out = nc.dram_tensor("out", [HID, QPC], F32, kind="ExternalOutput")

    act = _act_fn()

    with tile.TileContext(nc) as tc, ExitStack() as ctx:
        consts = ctx.enter_context(tc.tile_pool(name="consts", bufs=1))
        sb = ctx.enter_context(tc.tile_pool(name="sb", bufs=6))
        # per-stage PSUM pools so PE can run ahead of ACT (8 banks total:
        # 2x2 + 1x2 + 1x2)
        ps_a0p = ctx.enter_context(tc.tile_pool(name="psa0", bufs=2, space="PSUM"))
        ps_z1p = ctx.enter_context(tc.tile_pool(name="psz1", bufs=1, space="PSUM"))
        ps_z2p = ctx.enter_context(tc.tile_pool(name="psz2", bufs=1, space="PSUM"))
        misc = ctx.enter_context(tc.tile_pool(name="misc", bufs=1))

        def cload(dram, shape, dtype, name):
            t = consts.tile(shape, dtype, tag=name)
            nc.sync.dma_start(t[:], dram[:])
            return t

        t_xq = cload(d_xq, [4, QPC], F32, "xq")
        t_xs = cload(d_xs, [4, N], F32, "xs")
        t_w10 = cload(d_w10, [1, HID], mdt, "w10")
        t_A = cload(d_A, [HID, HID], mdt, "A")
        t_Bm = cload(d_Bm, [HID, HID], mdt, "Bm")
        t_Cs = cload(d_Cs, [HID, HID], F32, "Cs")
        t_b10 = cload(d_b10, [HID, 1], F32, "b10")
        t_c1 = cload(d_c1, [HID, 1], F32, "c1")
        t_c2 = cload(d_c2, [HID, 1], F32, "c2")
        t_c3 = cload(d_c3, [HID, 1], F32, "c3")

        # ---- distances: d2 = xq^T xs (K=4), relu, sqrt ----
        ps_d = ps_a0p.tile([128, N], F32, tag="psa0")
        nc.tensor.matmul(ps_d[:], t_xq[:], t_xs[:], start=True, stop=True)
        d2_sb = misc.tile([128, N], F32, tag="d2")
        nc.scalar.activation(d2_sb[:], ps_d[:], AF.Relu)
        dist = misc.tile([128, N], mdt, tag="dist")
        nc.scalar.activation(dist[:], d2_sb[:], AF.Sqrt)

        # matmul operands must sit at base partition 0 — flatten dist rows
        # into partition-0 chunks via SBUF->SBUF DMA (CQ query rows each).
        CQ = 16
        dflat = ctx.enter_context(tc.tile_pool(name="dflat", bufs=2))

        # ---- per-query-pair fused MLP chain ----
        t_S = misc.tile([HID, QPC], F32, tag="S")
        W = N  # 512 free per query
        fl = None
        for p in range(QPC // QB):
            if (QB * p) % CQ == 0:
                c = (QB * p) // CQ
                fl = dflat.tile([1, CQ * W], mdt, tag="dflat")
                nc.sync.dma_start(fl[:], dist[CQ * c : CQ * (c + 1), :])
            ps_a0 = ps_a0p.tile([128, QB * W], F32, tag="psa0")
            for k in range(QB):
                q = (QB * p + k) % CQ
                nc.tensor.matmul(
                    ps_a0[:, k * W : (k + 1) * W],
                    _mm_ap(t_w10[:]),
                    _mm_ap(fl[0:1, q * W : (q + 1) * W]),
                    start=True,
                    stop=True,
                )
            a0 = sb.tile([128, QB * W], mdt, tag="a")
            nc.scalar.activation(a0[:], ps_a0[:], act, bias=t_b10[:])

            ps_z1 = ps_z1p.tile([128, QB * W], F32, tag="psz1")
            for k in range(QB):
                nc.tensor.matmul(
                    ps_z1[:, k * W : (k + 1) * W],
                    _mm_ap(t_A[:]),
                    _mm_ap(a0[:, k * W : (k + 1) * W]),
                    start=True,
                    stop=True,
                )
            a1 = sb.tile([128, QB * W], mdt, tag="a")
            nc.scalar.activation(a1[:], ps_z1[:], act, bias=t_c1[:])

            ps_z2 = ps_z2p.tile([128, QB * W], F32, tag="psz2")
            for k in range(QB):
                nc.tensor.matmul(
                    ps_z2[:, k * W : (k + 1) * W],
                    _mm_ap(t_Bm[:]),
                    _mm_ap(a1[:, k * W : (k + 1) * W]),
                    start=True,
                    stop=True,
                )
            a2 = sb.tile([128, QB * W], F32, tag="a2")
            nc.scalar.activation(a2[:], ps_z2[:], act, bias=t_c2[:])

            nc.vector.reduce_sum(
                t_S[:, QB * p : QB * (p + 1)],
                a2[:].rearrange("h (q j) -> h q j", q=QB),
                axis=mybir.AxisListType.X,
            )

        # ---- final projection: out[o, i] = sum_h Cs[h, o] S[h, i] + c3[o] ----
        ps_o = ps_z2p.tile([HID, QPC], F32, tag="psz2")
        nc.tensor.matmul(ps_o[:], t_Cs[:], t_S[:], start=True, stop=True)
        out_sb = misc.tile([HID, QPC], F32, tag="out")
        nc.scalar.activation(out_sb[:], ps_o[:], AF.Identity, bias=t_c3[:])
        nc.sync.dma_start(d_out[:], out_sb[:])

    nc.compile()
    return nc


def build_nc_sym():
    """Symmetric-pair sharding kernel (SPMD-uniform shapes).

    Per core (batch b = c//4, block k = c%4, node blocks I0..I3 of 128):
      J12: queries I_k, sources [I_k | I_{k+1}]  (128 q x 256 j)
           rowsum -> S[:, 0:128]; colsum over j in [128:256) -> S[:, 128:256]
      J3:  64 queries x 128 sources (covers the {k, k+2} block-pair half)
           rowsum -> S[:, 256:320]; colsum -> S[:, 320:448]
    Device emits out_p = Cs^T @ S (128 o x 448 partial-query cols); the host
    gather adds partials into global query rows and adds c3 once.
    """
    nc = bacc.Bacc("TRN2", target_bir_lowering=False)
    mdt = _mm_tile_dt()

    d_xqA = nc.dram_tensor("xqA", [4, 128], F32, kind="ExternalInput")
    d_xsA = nc.dram_tensor("xsA", [4, 256], F32, kind="ExternalInput")
    d_xqB = nc.dram_tensor("xqB", [4, 64], F32, kind="ExternalInput")
    d_xsB = nc.dram_tensor("xsB", [4, 128], F32, kind="ExternalInput")
    d_w10 = nc.dram_tensor("w10", [1, HID], mdt, kind="ExternalInput")
    d_A = nc.dram_tensor("A", [HID, HID], mdt, kind="ExternalInput")
    d_Bm = nc.dram_tensor("Bm", [HID, HID], mdt, kind="ExternalInput")
    d_Cs = nc.dram_tensor("Cs", [HID, HID], F32, kind="ExternalInput")
    d_b10 = nc.dram_tensor("b10", [HID, 1], F32, kind="ExternalInput")
    d_c1 = nc.dram_tensor("c1", [HID, 1], F32, kind="ExternalInput")
    d_c2 = nc.dram_tensor("c2", [HID, 1], F32, kind="ExternalInput")
    d_out = nc.dram_tensor("out", [HID, 448], F32, kind="ExternalOutput")

    act = _act_fn()

    with tile.TileContext(nc) as tc, ExitStack() as ctx:
        consts = ctx.enter_context(tc.tile_pool(name="consts", bufs=1))
        sb = ctx.enter_context(tc.tile_pool(name="sb", bufs=8))
        ps_a0p = ctx.enter_context(tc.tile_pool(name="psa0", bufs=2, space="PSUM"))
        ps_z1p = ctx.enter_context(tc.tile_pool(name="psz1", bufs=1, space="PSUM"))
        ps_z2p = ctx.enter_context(tc.tile_pool(name="psz2", bufs=1, space="PSUM"))
        misc = ctx.enter_context(tc.tile_pool(name="misc", bufs=1))
        ctmps = ctx.enter_context(tc.tile_pool(name="ctmps", bufs=3))
        dflat = ctx.enter_context(tc.tile_pool(name="dflat", bufs=3))

        _eng = [nc.sync, nc.gpsimd, nc.scalar]
        _ei = [0]

        def cload(dram, shape, dtype, name):
            t = consts.tile(shape, dtype, tag=name)
            _eng[_ei[0] % len(_eng)].dma_start(t[:], dram[:])
            _ei[0] += 1
            return t

        t_xqA = cload(d_xqA, [4, 128], F32, "xqA")
        t_xsA = cload(d_xsA, [4, 256], F32, "xsA")
        t_xqB = cload(d_xqB, [4, 64], F32, "xqB")
        t_xsB = cload(d_xsB, [4, 128], F32, "xsB")
        t_w10 = cload(d_w10, [1, HID], mdt, "w10")
        t_A = cload(d_A, [HID, HID], mdt, "A")
        t_Bm = cload(d_Bm, [HID, HID], mdt, "Bm")
        t_Cs = cload(d_Cs, [HID, HID], F32, "Cs")
        t_b10 = cload(d_b10, [HID, 1], F32, "b10")
        t_c1 = cload(d_c1, [HID, 1], F32, "c1")
        t_c2 = cload(d_c2, [HID, 1], F32, "c2")

        # ---- PE warmup: ~6us of back-to-back dummy matmuls during the
        # (otherwise PE-idle) preamble locks the HAM clock gate at 2.4GHz;
        # it never re-throttles because steady-state PE gaps stay < 3.4us.
        wm = ps_z2p.tile([128, 128], F32, tag="psz2")
        for _ in range(56):
            nc.tensor.matmul(wm[:], t_A[:], t_A[:], start=True, stop=True)

        # ---- distances ----
        ps_d1 = ps_a0p.tile([128, 256], F32, tag="psa0")
        nc.tensor.matmul(ps_d1[:], t_xqA[:], t_xsA[:], start=True, stop=True)
        d2a = misc.tile([128, 256], F32, tag="d2a")
        nc.vector.tensor_scalar_max(d2a[:], ps_d1[:], 0.0)
        dist1 = misc.tile([128, 256], mdt, tag="dist1")
        nc.scalar.activation(dist1[:], d2a[:], AF.Sqrt)

        ps_d2 = ps_z1p.tile([64, 128], F32, tag="psz1")
        nc.tensor.matmul(ps_d2[:], t_xqB[:], t_xsB[:], start=True, stop=True)
        d2b = misc.tile([64, 128], F32, tag="d2b")
        nc.vector.tensor_scalar_max(d2b[:], ps_d2[:], 0.0)
        dist2 = misc.tile([64, 128], mdt, tag="dist2")
        nc.scalar.activation(dist2[:], d2b[:], AF.Sqrt)

        t_S = misc.tile([HID, 448], F32, tag="S")
        nc.gpsimd.memset(t_S[:, 128:256], 0.0)
        nc.gpsimd.memset(t_S[:, 320:448], 0.0)

        def chain(ps_a0, nq, w):
            """Silu(m0) -> matmul A -> Silu -> matmul Bm -> Silu; returns a2."""
            fd = nq * w
            a0 = sb.tile([128, fd], mdt, tag="a")
            nc.scalar.activation(a0[:], ps_a0[:], act, bias=t_b10[:])
            ps_z1 = ps_z1p.tile([128, fd], F32, tag="psz1")
            for k in range(0, fd, 512):
                nc.tensor.matmul(
                    ps_z1[:, k : k + 512], t_A[:], a0[:, k : k + 512],
                    start=True, stop=True,
                )
            a1 = sb.tile([128, fd], mdt, tag="a")
            nc.scalar.activation(a1[:], ps_z1[:], act, bias=t_c1[:])
            ps_z2 = ps_z2p.tile([128, fd], F32, tag="psz2")
            for k in range(0, fd, 512):
                nc.tensor.matmul(
                    ps_z2[:, k : k + 512], t_Bm[:], a1[:, k : k + 512],
                    start=True, stop=True,
                )
            a2 = sb.tile([128, fd], F32, tag="a2")
            nc.scalar.activation(a2[:], ps_z2[:], act, bias=t_c2[:])
            return a2

        # ---- J3: 8 groups of 8 queries x 128 sources ----
        fl3 = dflat.tile([1, 64 * 128], mdt, tag="dflat")
        nc.sync.dma_start(fl3[:], dist2[:, :])
        for p in range(8):
            ps_a0 = ps_a0p.tile([128, 1024], F32, tag="psa0")
            for k in range(8):
                q = 8 * p + k
                nc.tensor.matmul(
                    ps_a0[:, k * 128 : (k + 1) * 128],
                    t_w10[:],
                    fl3[0:1, q * 128 : (q + 1) * 128],
                    start=True, stop=True,
                )
            a2 = chain(ps_a0, 8, 128)
            v = a2[:].rearrange("h (q j) -> h q j", q=8)
            nc.vector.reduce_sum(
                t_S[:, 256 + 8 * p : 256 + 8 * (p + 1)], v,
                axis=mybir.AxisListType.X,
            )
            ctmp = ctmps.tile([128, 128], F32, tag="ctmp")
            nc.vector.reduce_sum(
                ctmp[:], v.transpose([0, 2, 1]), axis=mybir.AxisListType.X
            )
            nc.gpsimd.tensor_add(t_S[:, 320:448], t_S[:, 320:448], ctmp[:])

        # ---- partial projection for J3 columns (J3 ran first) ----
        out_sb = misc.tile([HID, 448], F32, tag="out")
        ps_o2 = ps_z2p.tile([HID, 192], F32, tag="psz2")
        nc.tensor.matmul(ps_o2[:], t_Cs[:], t_S[:, 256:448], start=True, stop=True)
        nc.scalar.copy(out_sb[:, 256:448], ps_o2[:])
        nc.sync.dma_start(d_out[:, 256:448], out_sb[:, 256:448])

        # ---- J12: 32 groups of 4 queries x 256 sources ----
        CQ1 = 16
        fl = None
        for p in range(32):
            if (4 * p) % CQ1 == 0:
                c = (4 * p) // CQ1
                fl = dflat.tile([1, CQ1 * 256], mdt, tag="dflat")
                nc.sync.dma_start(fl[:], dist1[CQ1 * c : CQ1 * (c + 1), :])
            ps_a0 = ps_a0p.tile([128, 1024], F32, tag="psa0")
            for k in range(4):
                q = (4 * p + k) % CQ1
                nc.tensor.matmul(
                    ps_a0[:, k * 256 : (k + 1) * 256],
                    t_w10[:],
                    fl[0:1, q * 256 : (q + 1) * 256],
                    start=True, stop=True,
                )
            a2 = chain(ps_a0, 4, 256)
            v = a2[:].rearrange("h (q j) -> h q j", q=4)
            nc.vector.reduce_sum(
                t_S[:, 4 * p : 4 * (p + 1)], v, axis=mybir.AxisListType.X
            )
            ctmp = ctmps.tile([128, 128], F32, tag="ctmp")
            nc.vector.reduce_sum(
                ctmp[:], v[:, :, 128:256].transpose([0, 2, 1]),
                axis=mybir.AxisListType.X,
            )
            nc.gpsimd.tensor_add(t_S[:, 128:256], t_S[:, 128:256], ctmp[:])

        # ---- partial projection for J12 columns ----
        ps_o = ps_z2p.tile([HID, 256], F32, tag="psz2")
        nc.tensor.matmul(ps_o[:], t_Cs[:], t_S[:, 0:256], start=True, stop=True)
        nc.scalar.copy(out_sb[:, 0:256], ps_o[:])
        nc.sync.dma_start(d_out[:, 0:256], out_sb[:, 0:256])

    nc.compile()
    return nc


def _aug_q(x0, x1, nrm):
    return np.stack([x0, x1, nrm, np.ones_like(x0)]).astype(np.float32)


def _aug_s(x0, x1, nrm):
    return np.stack([-2.0 * x0, -2.0 * x1, np.ones_like(x0), nrm]).astype(np.float32)


def _sym_blocks(k):
    """Returns (Q3, S3) local-node index arrays for core block k."""
    I = [np.arange(128 * m, 128 * (m + 1)) for m in range(4)]
    if k < 2:
        return I[k + 2][:64], I[k]
    return I[k][64:], I[(k + 2) % 4]


def make_in_maps_sym(inputs):
    w = fold_weights(inputs)
    mdt_np = ml_dtypes.bfloat16 if MM_MODE == "bf16" else np.float32
    shared = {
        "w10": w["w10"].astype(mdt_np),
        "A": w["A"].astype(mdt_np),
        "Bm": w["Bm"].astype(mdt_np),
        "Cs": w["Cs"].astype(np.float32),
        "b10": w["b10"].astype(np.float32),
        "c1": w["c1"].reshape(HID, 1).astype(np.float32),
        "c2": w["c2"].reshape(HID, 1).astype(np.float32),
    }
    x = np.asarray(inputs["x"], np.float32)
    nrm = x[..., 0] ** 2 + x[..., 1] ** 2
    in_maps = []
    for c in range(N_CORES):
        b, k = c // 4, c % 4
        Ik = np.arange(128 * k, 128 * (k + 1))
        Sa = np.concatenate([Ik, (Ik + 128) % 512 if k == 3 else
                             np.arange(128 * (k + 1), 128 * (k + 2))])
        Q3, S3 = _sym_blocks(k)
        x0, x1, nr = x[b, :, 0], x[b, :, 1], nrm[b]
        in_maps.append({
            "xqA": _aug_q(x0[Ik], x1[Ik], nr[Ik]),
            "xsA": _aug_s(x0[Sa], x1[Sa], nr[Sa]),
            "xqB": _aug_q(x0[Q3], x1[Q3], nr[Q3]),
            "xsB": _aug_s(x0[S3], x1[S3], nr[S3]),
            **shared,
        })
    return in_maps


def combine_sym(outs, inputs):
    """outs: list of 8 per-core (128 o, 448) partial arrays."""
    w = fold_weights(inputs)
    out = np.broadcast_to(
        w["c3"].astype(np.float32), (B, N, HID)
    ).copy()
    for c in range(N_CORES):
        b, k = c // 4, c % 4
        P = outs[c]
        Ik = np.arange(128 * k, 128 * (k + 1))
        Inext = (Ik + 128) % 512
        Q3, S3 = _sym_blocks(k)
        out[b, Ik, :] += P[:, 0:128].T
        out[b, Inext, :] += P[:, 128:256].T
        out[b, Q3, :] += P[:, 256:320].T
        out[b, S3, :] += P[:, 320:448].T
    return out


def fold_weights(inputs):
    f64 = {k: np.asarray(v, np.float64) for k, v in inputs.items()}
    out = {}
    out["A"] = f64["W2_0"] @ f64["W1_1"]
    out["c1"] = f64["b2_0"] @ f64["W1_1"] + f64["b1_1"]
    out["Bm"] = f64["W2_1"] @ f64["W1_2"]
    out["c2"] = f64["b2_1"] @ f64["W1_2"] + f64["b1_2"]
    out["Cs"] = (f64["W2_2"] @ f64["Wo"]) / float(N)
    out["c3"] = f64["b2_2"] @ f64["Wo"] + f64["bo"]
    out["w10"] = f64["W1_0"].reshape(1, HID)
    out["b10"] = f64["b1_0"].reshape(HID, 1)
    return out


def make_in_maps(inputs):
    w = fold_weights(inputs)
    mdt_np = ml_dtypes.bfloat16 if MM_MODE == "bf16" else np.float32
    shared = {
        "w10": w["w10"].astype(mdt_np),
        "A": w["A"].astype(mdt_np),
        "Bm": w["Bm"].astype(mdt_np),
        "Cs": w["Cs"].astype(np.float32),
        "b10": w["b10"].astype(np.float32),
        "c1": w["c1"].reshape(HID, 1).astype(np.float32),
        "c2": w["c2"].reshape(HID, 1).astype(np.float32),
        "c3": w["c3"].reshape(HID, 1).astype(np.float32),
    }
    x = np.asarray(inputs["x"], np.float32)  # (B, N, 2)
    nrm = x[..., 0] ** 2 + x[..., 1] ** 2  # (B, N)
    in_maps = []
    for c in range(N_CORES):
        b = c // (N_CORES // B)
        i0 = (c % (N_CORES // B)) * QPC
        xq = np.stack(
            [
                x[b, i0 : i0 + QPC, 0],
                x[b, i0 : i0 + QPC, 1],
                nrm[b, i0 : i0 + QPC],
                np.ones(QPC, np.float32),
            ]
        ).astype(np.float32)
        xs = np.stack(
            [
                -2.0 * x[b, :, 0],
                -2.0 * x[b, :, 1],
                np.ones(N, np.float32),
                nrm[b, :],
            ]
        ).astype(np.float32)
        in_maps.append({"xq": xq, "xs": xs, **shared})
    return in_maps


_NC_CACHE = {}


def get_nc():
    key = (MM_MODE, ACT_NAME, QB, ALGO)
    if key not in _NC_CACHE:
        _NC_CACHE[key] = build_nc_sym() if ALGO == "sym" else build_nc()
    return _NC_CACHE[key]


def run(inputs, trace=False, tmpdir=None):
    """Run on 8 cores; returns (full_output, BassKernelResults)."""
    nc = get_nc()
    if ALGO == "sym":
        in_maps = make_in_maps_sym(inputs)
    else:
        in_maps = make_in_maps(inputs)
    try:
        res = run_bass_kernel_spmd(
            nc, in_maps, list(range(N_CORES)), trace=trace, tmpdir=tmpdir
        )
    except Exception:
        # transient NRT device errors (e.g. NRT_EXEC_UNIT_UNRECOVERABLE from a
        # prior wedged run) usually clear on retry
        res = run_bass_kernel_spmd(
            nc, in_maps, list(range(N_CORES)), trace=trace, tmpdir=tmpdir
        )
    if ALGO == "sym":
        return combine_sym([res.results[c]["out"] for c in range(N_CORES)],
                           inputs), res
    out = np.empty((B, N, HID), np.float32)
    for c in range(N_CORES):
        b = c // (N_CORES // B)
        i0 = (c % (N_CORES // B)) * QPC
        out[b, i0 : i0 + QPC, :] = res.results[c]["out"].T
    return out, res


def kernel(**inputs):
    out, _ = run(inputs)
    return out


# revision 16
# speedup vs baseline: 1.0370x; 1.0370x over previous
"""Trainium2 Bass kernel for nn_DistanceEncoder (gnn_message_passing).

Reference math (per batch b of 2, n=512 nodes, hid=128):
  dist = cdist(x, x)                               (n, n)
  h    = MLP0(dist[..., None]); h = MLP1(h); h = MLP2(h)   per-edge (n, n, 128)
  out  = mean_j(h) @ Wo + bo                       (n, 128)

Host-side algebraic folding (exact up to fp rounding):
  a0 = SiLU(d * w1_0 + b1_0)
  t1 = a0 @ A + c1,  A = W2_0 @ W1_1,  c1 = b2_0 @ W1_1 + b1_1
  a1 = SiLU(t1)
  t2 = a1 @ Bm + c2, Bm = W2_1 @ W1_2, c2 = b2_1 @ W1_2 + b1_2
  a2 = SiLU(t2)
  S_i = sum_j a2_ij
  out_i = S_i @ Cs + c3,  Cs = (W2_2 @ Wo)/512,  c3 = b2_2 @ Wo + bo

Sharding: the 2*512=1024 query rows are split across 8 cores (128 each; cores
0-3 handle batch 0, cores 4-7 batch 1). Each core sees all 512 sources of its
batch; the mean-aggregation is local, no collectives.

Distances are computed on the PE from augmented features:
  d2[i, j] = x_i . x_i + x_j . x_j - 2 x_i . x_j  (K=4 matmul), then relu+sqrt.
"""

import os
from contextlib import ExitStack

import numpy as np
import ml_dtypes

import concourse.bacc as bacc
import concourse.bass as bass
import concourse.mybir as mybir
import concourse.tile as tile
from concourse.bass_utils import run_bass_kernel_spmd

N_CORES = 8
B, N, HID = 2, 512, 128
QPC = (B * N) // N_CORES  # 128 queries per core
F32 = mybir.dt.float32
AF = mybir.ActivationFunctionType

# matmul precision mode: "f32r" (fp32 data, reduced-precision PE pass at full
# rate), "bf16", or "f32" (4x slower PE)
MM_MODE = os.environ.get("DE_MM_MODE", "bf16")
# "sym": symmetric pair sharding (each edge computed once globally, partial
# sums combined during the host gather); "basic": plain query sharding.
ALGO = os.environ.get("DE_ALGO", "sym")
# activation used on device; Silu is the real one. "sigmoid" only for CoreSim
# debugging (the python interp lacks Silu).
ACT_NAME = os.environ.get("DE_ACT", "silu")
QB = 2  # queries batched per activation instruction


def _act_fn():
    return AF.Silu if ACT_NAME == "silu" else AF.Sigmoid


def _mm_tile_dt():
    if MM_MODE == "bf16":
        return mybir.dt.bfloat16
    if MM_MODE == "f32r":
        return mybir.dt.float32r
    return F32


def _mm_ap(ap):
    return ap


def build_nc():
    nc = bacc.Bacc("TRN2", target_bir_lowering=False)
    mdt = _mm_tile_dt()

    # DRAM I/O (per-core shapes)
    d_xq = nc.dram_tensor("xq", [4, QPC], F32, kind="ExternalInput")
    d_xs = nc.dram_tensor("xs", [4, N], F32, kind="ExternalInput")
    d_w10 = nc.dram_tensor("w10", [1, HID], mdt, kind="ExternalInput")
    d_A = nc.dram_tensor("A", [HID, HID], mdt, kind="ExternalInput")
    d_Bm = nc.dram_tensor("Bm", [HID, HID], mdt, kind="ExternalInput")
    d_Cs = nc.dram_tensor("Cs", [HID, HID], F32, kind="ExternalInput")
    d_b10 = nc.dram_tensor("b10", [HID, 1], F32, kind="ExternalInput")
    d_c1 = nc.dram_tensor("c1", [HID, 1], F32, kind="ExternalInput")
    d_c2 = nc.dram_tensor("c2", [HID, 1], F32, kind="ExternalInput")
    d_c3 = nc.dram_tensor("c3", [HID, 1], F32, kind="ExternalInput")
    d_out = nc.dram_tensor("out", [HID, QPC], F32, kind="ExternalOutput")

    act = _act_fn()

    with tile.TileContext(nc) as tc, ExitStack() as ctx:
        consts = ctx.enter_context(tc.tile_pool(name="consts", bufs=1))
        sb = ctx.enter_context(tc.tile_pool(name="sb", bufs=6))
        # per-stage PSUM pools so PE can run ahead of ACT (8 banks total:
        # 2x2 + 1x2 + 1x2)
        ps_a0p = ctx.enter_context(tc.tile_pool(name="psa0", bufs=2, space="PSUM"))
        ps_z1p = ctx.enter_context(tc.tile_pool(name="psz1", bufs=1, space="PSUM"))
        ps_z2p = ctx.enter_context(tc.tile_pool(name="psz2", bufs=1, space="PSUM"))
        misc = ctx.enter_context(tc.tile_pool(name="misc", bufs=1))

        def cload(dram, shape, dtype, name):
            t = consts.tile(shape, dtype, tag=name)
            nc.sync.dma_start(t[:], dram[:])
            return t

        t_xq = cload(d_xq, [4, QPC], F32, "xq")
        t_xs = cload(d_xs, [4, N], F32, "xs")
        t_w10 = cload(d_w10, [1, HID], mdt, "w10")
        t_A = cload(d_A, [HID, HID], mdt, "A")
        t_Bm = cload(d_Bm, [HID, HID], mdt, "Bm")
        t_Cs = cload(d_Cs, [HID, HID], F32, "Cs")
        t_b10 = cload(d_b10, [HID, 1], F32, "b10")
        t_c1 = cload(d_c1, [HID, 1], F32, "c1")
        t_c2 = cload(d_c2, [HID, 1], F32, "c2")
        t_c3 = cload(d_c3, [HID, 1], F32, "c3")

        # ---- distances: d2 = xq^T xs (K=4), relu, sqrt ----
        ps_d = ps_a0p.tile([128, N], F32, tag="psa0")
        nc.tensor.matmul(ps_d[:], t_xq[:], t_xs[:], start=True, stop=True)
        d2_sb = misc.tile([128, N], F32, tag="d2")
        nc.scalar.activation(d2_sb[:], ps_d[:], AF.Relu)
        dist = misc.tile([128, N], mdt, tag="dist")
        nc.scalar.activation(dist[:], d2_sb[:], AF.Sqrt)

        # matmul operands must sit at base partition 0 — flatten dist rows
        # into partition-0 chunks via SBUF->SBUF DMA (CQ query rows each).
        CQ = 16
        dflat = ctx.enter_context(tc.tile_pool(name="dflat", bufs=2))

        # ---- per-query-pair fused MLP chain ----
        t_S = misc.tile([HID, QPC], F32, tag="S")
        W = N  # 512 free per query
        fl = None
        for p in range(QPC // QB):
            if (QB * p) % CQ == 0:
                c = (QB * p) // CQ
                fl = dflat.tile([1, CQ * W], mdt, tag="dflat")
                nc.sync.dma_start(fl[:], dist[CQ * c : CQ * (c + 1), :])
            ps_a0 = ps_a0p.tile([128, QB * W], F32, tag="psa0")
            for k in range(QB):
                q = (QB * p + k) % CQ
                nc.tensor.matmul(
                    ps_a0[:, k * W : (k + 1) * W],
                    _mm_ap(t_w10[:]),
                    _mm_ap(fl[0:1, q * W : (q + 1) * W]),
                    start=True,
                    stop=True,
                )
            a0 = sb.tile([128, QB * W], mdt, tag="a")
            nc.scalar.activation(a0[:], ps_a0[:], act, bias=t_b10[:])

            ps_z1 = ps_z1p.tile([128, QB * W], F32, tag="psz1")
            for k in range(QB):
                nc.tensor.matmul(
                    ps_z1[:, k * W : (k + 1) * W],
                    _mm_ap(t_A[:]),
                    _mm_ap(a0[:, k * W : (k + 1) * W]),
                    start=True,
                    stop=True,
                )
            a1 = sb.tile([128, QB * W], mdt, tag="a")
            nc.scalar.activation(a1[:], ps_z1[:], act, bias=t_c1[:])

            ps_z2 = ps_z2p.tile([128, QB * W], F32, tag="psz2")
            for k in range(QB):
                nc.tensor.matmul(
                    ps_z2[:, k * W : (k + 1) * W],
                    _mm_ap(t_Bm[:]),
                    _mm_ap(a1[:, k * W : (k + 1) * W]),
                    start=True,
                    stop=True,
                )
            a2 = sb.tile([128, QB * W], F32, tag="a2")
            nc.scalar.activation(a2[:], ps_z2[:], act, bias=t_c2[:])

            nc.vector.reduce_sum(
                t_S[:, QB * p : QB * (p + 1)],
                a2[:].rearrange("h (q j) -> h q j", q=QB),
                axis=mybir.AxisListType.X,
            )

        # ---- final projection: out[o, i] = sum_h Cs[h, o] S[h, i] + c3[o] ----
        ps_o = ps_z2p.tile([HID, QPC], F32, tag="psz2")
        nc.tensor.matmul(ps_o[:], t_Cs[:], t_S[:], start=True, stop=True)
        out_sb = misc.tile([HID, QPC], F32, tag="out")
        nc.scalar.activation(out_sb[:], ps_o[:], AF.Identity, bias=t_c3[:])
        nc.sync.dma_start(d_out[:], out_sb[:])

    nc.compile()
    return nc


def build_nc_sym():
    """Symmetric-pair sharding kernel (SPMD-uniform shapes).

    Per core (batch b = c//4, block k = c%4, node blocks I0..I3 of 128):
      J12: queries I_k, sources [I_k | I_{k+1}]  (128 q x 256 j)
           rowsum -> S[:, 0:128]; colsum over j in [128:256) -> S[:, 128:256]
      J3:  64 queries x 128 sources (covers the {k, k+2} block-pair half)
           rowsum -> S[:, 256:320]; colsum -> S[:, 320:448]
    Device emits out_p = Cs^T @ S (128 o x 448 partial-query cols); the host
    gather adds partials into global query rows and adds c3 once.
    """
    nc = bacc.Bacc("TRN2", target_bir_lowering=False)
    mdt = _mm_tile_dt()

    d_xqA = nc.dram_tensor("xqA", [4, 128], F32, kind="ExternalInput")
    d_xsA = nc.dram_tensor("xsA", [4, 256], F32, kind="ExternalInput")
    d_xqB = nc.dram_tensor("xqB", [4, 64], F32, kind="ExternalInput")
    d_xsB = nc.dram_tensor("xsB", [4, 128], F32, kind="ExternalInput")
    d_w10 = nc.dram_tensor("w10", [1, HID], mdt, kind="ExternalInput")
    d_A = nc.dram_tensor("A", [HID, HID], mdt, kind="ExternalInput")
    d_Bm = nc.dram_tensor("Bm", [HID, HID], mdt, kind="ExternalInput")
    d_Cs = nc.dram_tensor("Cs", [HID, HID], F32, kind="ExternalInput")
    d_b10 = nc.dram_tensor("b10", [HID, 1], F32, kind="ExternalInput")
    d_c1 = nc.dram_tensor("c1", [HID, 1], F32, kind="ExternalInput")
    d_c2 = nc.dram_tensor("c2", [HID, 1], F32, kind="ExternalInput")
    d_out = nc.dram_tensor("out", [HID, 448], F32, kind="ExternalOutput")

    act = _act_fn()

    with tile.TileContext(nc) as tc, ExitStack() as ctx:
        consts = ctx.enter_context(tc.tile_pool(name="consts", bufs=1))
        sb = ctx.enter_context(tc.tile_pool(name="sb", bufs=8))
        ps_a0p = ctx.enter_context(tc.tile_pool(name="psa0", bufs=2, space="PSUM"))
        ps_z1p = ctx.enter_context(tc.tile_pool(name="psz1", bufs=1, space="PSUM"))
        ps_z2p = ctx.enter_context(tc.tile_pool(name="psz2", bufs=1, space="PSUM"))
        misc = ctx.enter_context(tc.tile_pool(name="misc", bufs=1))
        ctmps = ctx.enter_context(tc.tile_pool(name="ctmps", bufs=3))
        dflat = ctx.enter_context(tc.tile_pool(name="dflat", bufs=3))

        _eng = [nc.sync, nc.gpsimd, nc.scalar]
        _ei = [0]

        def cload(dram, shape, dtype, name):
            t = consts.tile(shape, dtype, tag=name)
            _eng[_ei[0] % len(_eng)].dma_start(t[:], dram[:])
            _ei[0] += 1
            return t

        t_xqA = cload(d_xqA, [4, 128], F32, "xqA")
        t_xsA = cload(d_xsA, [4, 256], F32, "xsA")
        t_xqB = cload(d_xqB, [4, 64], F32, "xqB")
        t_xsB = cload(d_xsB, [4, 128], F32, "xsB")
        t_w10 = cload(d_w10, [1, HID], mdt, "w10")
        t_A = cload(d_A, [HID, HID], mdt, "A")
        t_Bm = cload(d_Bm, [HID, HID], mdt, "Bm")
        t_Cs = cload(d_Cs, [HID, HID], F32, "Cs")
        t_b10 = cload(d_b10, [HID, 1], F32, "b10")
        t_c1 = cload(d_c1, [HID, 1], F32, "c1")
        t_c2 = cload(d_c2, [HID, 1], F32, "c2")

        # ---- distances ----
        ps_d1 = ps_a0p.tile([128, 256], F32, tag="psa0")
        nc.tensor.matmul(ps_d1[:], t_xqA[:], t_xsA[:], start=True, stop=True)
        d2a = misc.tile([128, 256], F32, tag="d2a")
        nc.vector.tensor_scalar_max(d2a[:], ps_d1[:], 0.0)
        dist1 = misc.tile([128, 256], mdt, tag="dist1")
        nc.scalar.activation(dist1[:], d2a[:], AF.Sqrt)

        ps_d2 = ps_z1p.tile([64, 128], F32, tag="psz1")
        nc.tensor.matmul(ps_d2[:], t_xqB[:], t_xsB[:], start=True, stop=True)
        d2b = misc.tile([64, 128], F32, tag="d2b")
        nc.vector.tensor_scalar_max(d2b[:], ps_d2[:], 0.0)
        dist2 = misc.tile([64, 128], mdt, tag="dist2")
        nc.scalar.activation(dist2[:], d2b[:], AF.Sqrt)

        t_S = misc.tile([HID, 448], F32, tag="S")
        nc.gpsimd.memset(t_S[:, 128:256], 0.0)
        nc.gpsimd.memset(t_S[:, 320:448], 0.0)

        def chain(ps_a0, nq, w):
            """Silu(m0) -> matmul A -> Silu -> matmul Bm -> Silu; returns a2."""
            fd = nq * w
            a0 = sb.tile([128, fd], mdt, tag="a")
            nc.scalar.activation(a0[:], ps_a0[:], act, bias=t_b10[:])
            ps_z1 = ps_z1p.tile([128, fd], F32, tag="psz1")
            for k in range(0, fd, 512):
                nc.tensor.matmul(
                    ps_z1[:, k : k + 512], t_A[:], a0[:, k : k + 512],
                    start=True, stop=True,
                )
            a1 = sb.tile([128, fd], mdt, tag="a")
            nc.scalar.activation(a1[:], ps_z1[:], act, bias=t_c1[:])
            ps_z2 = ps_z2p.tile([128, fd], F32, tag="psz2")
            for k in range(0, fd, 512):
                nc.tensor.matmul(
                    ps_z2[:, k : k + 512], t_Bm[:], a1[:, k : k + 512],
                    start=True, stop=True,
                )
            a2 = sb.tile([128, fd], F32, tag="a2")
            nc.scalar.activation(a2[:], ps_z2[:], act, bias=t_c2[:])
            return a2

        # ---- J3: 8 groups of 8 queries x 128 sources ----
        fl3 = dflat.tile([1, 64 * 128], mdt, tag="dflat")
        nc.sync.dma_start(fl3[:], dist2[:, :])
        for p in range(8):
            ps_a0 = ps_a0p.tile([128, 1024], F32, tag="psa0")
            for k in range(2):
                nc.tensor.matmul(
                    ps_a0[:, k * 512 : (k + 1) * 512],
                    t_w10[:],
                    fl3[0:1, p * 1024 + k * 512 : p * 1024 + (k + 1) * 512],
                    start=True, stop=True,
                )
            a2 = chain(ps_a0, 8, 128)
            v = a2[:].rearrange("h (q j) -> h q j", q=8)
            nc.vector.reduce_sum(
                t_S[:, 256 + 8 * p : 256 + 8 * (p + 1)], v,
                axis=mybir.AxisListType.X,
            )
            ctmp = ctmps.tile([128, 128], F32, tag="ctmp")
            nc.vector.reduce_sum(
                ctmp[:], v.transpose([0, 2, 1]), axis=mybir.AxisListType.X
            )
            nc.gpsimd.tensor_add(t_S[:, 320:448], t_S[:, 320:448], ctmp[:])

        # ---- partial projection for J3 columns (J3 ran first) ----
        out_sb = misc.tile([HID, 448], F32, tag="out")
        ps_o2 = ps_z2p.tile([HID, 192], F32, tag="psz2")
        nc.tensor.matmul(ps_o2[:], t_Cs[:], t_S[:, 256:448], start=True, stop=True)
        nc.scalar.copy(out_sb[:, 256:448], ps_o2[:])
        nc.sync.dma_start(d_out[:, 256:448], out_sb[:, 256:448])

        # ---- J12: 32 groups of 4 queries x 256 sources ----
        CQ1 = 16
        fl = None
        for p in range(32):
            if (4 * p) % CQ1 == 0:
                c = (4 * p) // CQ1
                fl = dflat.tile([1, CQ1 * 256], mdt, tag="dflat")
                nc.sync.dma_start(fl[:], dist1[CQ1 * c : CQ1 * (c + 1), :])
            ps_a0 = ps_a0p.tile([128, 1024], F32, tag="psa0")
            base = ((4 * p) % CQ1) * 256
            for k in range(2):
                nc.tensor.matmul(
                    ps_a0[:, k * 512 : (k + 1) * 512],
                    t_w10[:],
                    fl[0:1, base + k * 512 : base + (k + 1) * 512],
                    start=True, stop=True,
                )
            a2 = chain(ps_a0, 4, 256)
            v = a2[:].rearrange("h (q j) -> h q j", q=4)
            nc.vector.reduce_sum(
                t_S[:, 4 * p : 4 * (p + 1)], v, axis=mybir.AxisListType.X
            )
            ctmp = ctmps.tile([128, 128], F32, tag="ctmp")
            nc.vector.reduce_sum(
                ctmp[:], v[:, :, 128:256].transpose([0, 2, 1]),
                axis=mybir.AxisListType.X,
            )
            nc.gpsimd.tensor_add(t_S[:, 128:256], t_S[:, 128:256], ctmp[:])

        # ---- partial projection for J12 columns ----
        ps_o = ps_z2p.tile([HID, 256], F32, tag="psz2")
        nc.tensor.matmul(ps_o[:], t_Cs[:], t_S[:, 0:256], start=True, stop=True)
        nc.scalar.copy(out_sb[:, 0:256], ps_o[:])
        nc.sync.dma_start(d_out[:, 0:256], out_sb[:, 0:256])

    nc.compile()
    return nc


def _aug_q(x0, x1, nrm):
    return np.stack([x0, x1, nrm, np.ones_like(x0)]).astype(np.float32)


def _aug_s(x0, x1, nrm):
    return np.stack([-2.0 * x0, -2.0 * x1, np.ones_like(x0), nrm]).astype(np.float32)


def _sym_blocks(k):
    """Returns (Q3, S3) local-node index arrays for core block k."""
    I = [np.arange(128 * m, 128 * (m + 1)) for m in range(4)]
    if k < 2:
        return I[k + 2][:64], I[k]
    return I[k][64:], I[(k + 2) % 4]


def make_in_maps_sym(inputs):
    w = fold_weights(inputs)
    mdt_np = ml_dtypes.bfloat16 if MM_MODE == "bf16" else np.float32
    shared = {
        "w10": w["w10"].astype(mdt_np),
        "A": w["A"].astype(mdt_np),
        "Bm": w["Bm"].astype(mdt_np),
        "Cs": w["Cs"].astype(np.float32),
        "b10": w["b10"].astype(np.float32),
        "c1": w["c1"].reshape(HID, 1).astype(np.float32),
        "c2": w["c2"].reshape(HID, 1).astype(np.float32),
    }
    x = np.asarray(inputs["x"], np.float32)
    nrm = x[..., 0] ** 2 + x[..., 1] ** 2
    in_maps = []
    for c in range(N_CORES):
        b, k = c // 4, c % 4
        Ik = np.arange(128 * k, 128 * (k + 1))
        Sa = np.concatenate([Ik, (Ik + 128) % 512 if k == 3 else
                             np.arange(128 * (k + 1), 128 * (k + 2))])
        Q3, S3 = _sym_blocks(k)
        x0, x1, nr = x[b, :, 0], x[b, :, 1], nrm[b]
        in_maps.append({
            "xqA": _aug_q(x0[Ik], x1[Ik], nr[Ik]),
            "xsA": _aug_s(x0[Sa], x1[Sa], nr[Sa]),
            "xqB": _aug_q(x0[Q3], x1[Q3], nr[Q3]),
            "xsB": _aug_s(x0[S3], x1[S3], nr[S3]),
            **shared,
        })
    return in_maps


def combine_sym(outs, inputs):
    """outs: list of 8 per-core (128 o, 448) partial arrays."""
    w = fold_weights(inputs)
    out = np.broadcast_to(
        w["c3"].astype(np.float32), (B, N, HID)
    ).copy()
    for c in range(N_CORES):
        b, k = c // 4, c % 4
        P = outs[c]
        Ik = np.arange(128 * k, 128 * (k + 1))
        Inext = (Ik + 128) % 512
        Q3, S3 = _sym_blocks(k)
        out[b, Ik, :] += P[:, 0:128].T
        out[b, Inext, :] += P[:, 128:256].T
        out[b, Q3, :] += P[:, 256:320].T
        out[b, S3, :] += P[:, 320:448].T
    return out


def fold_weights(inputs):
    f64 = {k: np.asarray(v, np.float64) for k, v in inputs.items()}
    out = {}
    out["A"] = f64["W2_0"] @ f64["W1_1"]
    out["c1"] = f64["b2_0"] @ f64["W1_1"] + f64["b1_1"]
    out["Bm"] = f64["W2_1"] @ f64["W1_2"]
    out["c2"] = f64["b2_1"] @ f64["W1_2"] + f64["b1_2"]
    out["Cs"] = (f64["W2_2"] @ f64["Wo"]) / float(N)
    out["c3"] = f64["b2_2"] @ f64["Wo"] + f64["bo"]
    out["w10"] = f64["W1_0"].reshape(1, HID)
    out["b10"] = f64["b1_0"].reshape(HID, 1)
    return out


def make_in_maps(inputs):
    w = fold_weights(inputs)
    mdt_np = ml_dtypes.bfloat16 if MM_MODE == "bf16" else np.float32
    shared = {
        "w10": w["w10"].astype(mdt_np),
        "A": w["A"].astype(mdt_np),
        "Bm": w["Bm"].astype(mdt_np),
        "Cs": w["Cs"].astype(np.float32),
        "b10": w["b10"].astype(np.float32),
        "c1": w["c1"].reshape(HID, 1).astype(np.float32),
        "c2": w["c2"].reshape(HID, 1).astype(np.float32),
        "c3": w["c3"].reshape(HID, 1).astype(np.float32),
    }
    x = np.asarray(inputs["x"], np.float32)  # (B, N, 2)
    nrm = x[..., 0] ** 2 + x[..., 1] ** 2  # (B, N)
    in_maps = []
    for c in range(N_CORES):
        b = c // (N_CORES // B)
        i0 = (c % (N_CORES // B)) * QPC
        xq = np.stack(
            [
                x[b, i0 : i0 + QPC, 0],
                x[b, i0 : i0 + QPC, 1],
                nrm[b, i0 : i0 + QPC],
                np.ones(QPC, np.float32),
            ]
        ).astype(np.float32)
        xs = np.stack(
            [
                -2.0 * x[b, :, 0],
                -2.0 * x[b, :, 1],
                np.ones(N, np.float32),
                nrm[b, :],
            ]
        ).astype(np.float32)
        in_maps.append({"xq": xq, "xs": xs, **shared})
    return in_maps


_NC_CACHE = {}


def get_nc():
    key = (MM_MODE, ACT_NAME, QB, ALGO)
    if key not in _NC_CACHE:
        _NC_CACHE[key] = build_nc_sym() if ALGO == "sym" else build_nc()
    return _NC_CACHE[key]


def run(inputs, trace=False, tmpdir=None):
    """Run on 8 cores; returns (full_output, BassKernelResults)."""
    nc = get_nc()
    if ALGO == "sym":
        in_maps = make_in_maps_sym(inputs)
    else:
        in_maps = make_in_maps(inputs)
    try:
        res = run_bass_kernel_spmd(
            nc, in_maps, list(range(N_CORES)), trace=trace, tmpdir=tmpdir
        )
    except Exception:
        # transient NRT device errors (e.g. NRT_EXEC_UNIT_UNRECOVERABLE from a
        # prior wedged run) usually clear on retry
        res = run_bass_kernel_spmd(
            nc, in_maps, list(range(N_CORES)), trace=trace, tmpdir=tmpdir
        )
    if ALGO == "sym":
        return combine_sym([res.results[c]["out"] for c in range(N_CORES)],
                           inputs), res
    out = np.empty((B, N, HID), np.float32)
    for c in range(N_CORES):
        b = c // (N_CORES // B)
        i0 = (c % (N_CORES // B)) * QPC
        out[b, i0 : i0 + QPC, :] = res.results[c]["out"].T
    return out, res


def kernel(**inputs):
    out, _ = run(inputs)
    return out


# revision 18
# speedup vs baseline: 1.1013x; 1.0620x over previous
"""Trainium2 Bass kernel for nn_DistanceEncoder (gnn_message_passing).

Reference math (per batch b of 2, n=512 nodes, hid=128):
  dist = cdist(x, x)                               (n, n)
  h    = MLP0(dist[..., None]); h = MLP1(h); h = MLP2(h)   per-edge (n, n, 128)
  out  = mean_j(h) @ Wo + bo                       (n, 128)

Host-side algebraic folding (exact up to fp rounding):
  a0 = SiLU(d * w1_0 + b1_0)
  t1 = a0 @ A + c1,  A = W2_0 @ W1_1,  c1 = b2_0 @ W1_1 + b1_1
  a1 = SiLU(t1)
  t2 = a1 @ Bm + c2, Bm = W2_1 @ W1_2, c2 = b2_1 @ W1_2 + b1_2
  a2 = SiLU(t2)
  S_i = sum_j a2_ij
  out_i = S_i @ Cs + c3,  Cs = (W2_2 @ Wo)/512,  c3 = b2_2 @ Wo + bo

Sharding: the 2*512=1024 query rows are split across 8 cores (128 each; cores
0-3 handle batch 0, cores 4-7 batch 1). Each core sees all 512 sources of its
batch; the mean-aggregation is local, no collectives.

Distances are computed on the PE from augmented features:
  d2[i, j] = x_i . x_i + x_j . x_j - 2 x_i . x_j  (K=4 matmul), then relu+sqrt.
"""

import os
from contextlib import ExitStack

import numpy as np
import ml_dtypes

import concourse.bacc as bacc
import concourse.bass as bass
import concourse.mybir as mybir
import concourse.tile as tile
from concourse.bass_utils import run_bass_kernel_spmd

N_CORES = 8
B, N, HID = 2, 512, 128
QPC = (B * N) // N_CORES  # 128 queries per core
F32 = mybir.dt.float32
AF = mybir.ActivationFunctionType

# matmul precision mode: "f32r" (fp32 data, reduced-precision PE pass at full
# rate), "bf16", or "f32" (4x slower PE)
MM_MODE = os.environ.get("DE_MM_MODE", "bf16")
# "sym": symmetric pair sharding (each edge computed once globally, partial
# sums combined during the host gather); "basic": plain query sharding.
ALGO = os.environ.get("DE_ALGO", "sym")
# activation used on device; Silu is the real one. "sigmoid" only for CoreSim
# debugging (the python interp lacks Silu).
ACT_NAME = os.environ.get("DE_ACT", "silu")
QB = 2  # queries batched per activation instruction


def _act_fn():
    return AF.Silu if ACT_NAME == "silu" else AF.Sigmoid


def _mm_tile_dt():
    if MM_MODE == "bf16":
        return mybir.dt.bfloat16
    if MM_MODE == "f32r":
        return mybir.dt.float32r
    return F32


def _mm_ap(ap):
    return ap


def build_nc():
    nc = bacc.Bacc("TRN2", target_bir_lowering=False)
    mdt = _mm_tile_dt()

    # DRAM I/O (per-core shapes)
    d_xq = nc.dram_tensor("xq", [4, QPC], F32, kind="ExternalInput")
    d_xs = nc.dram_tensor("xs", [4, N], F32, kind="ExternalInput")
    d_w10 = nc.dram_tensor("w10", [1, HID], mdt, kind="ExternalInput")
    d_A = nc.dram_tensor("A", [HID, HID], mdt, kind="ExternalInput")
    d_Bm = nc.dram_tensor("Bm", [HID, HID], mdt, kind="ExternalInput")
    d_Cs = nc.dram_tensor("Cs", [HID, HID], F32, kind="ExternalInput")
    d_b10 = nc.dram_tensor("b10", [HID, 1], F32, kind="ExternalInput")
    d_c1 = nc.dram_tensor("c1", [HID, 1], F32, kind="ExternalInput")
    d_c2 = nc.dram_tensor("c2", [HID, 1], F32, kind="ExternalInput")
    d_c3 = nc.dram_tensor("c3", [HID, 1], F32, kind="ExternalInput")
    d_out = nc.dram_tensor("out", [HID, QPC], F32, kind="ExternalOutput")

    act = _act_fn()

    with tile.TileContext(nc) as tc, ExitStack() as ctx:
        consts = ctx.enter_context(tc.tile_pool(name="consts", bufs=1))
        sb = ctx.enter_context(tc.tile_pool(name="sb", bufs=6))
        # per-stage PSUM pools so PE can run ahead of ACT (8 banks total:
        # 2x2 + 1x2 + 1x2)
        ps_a0p = ctx.enter_context(tc.tile_pool(name="psa0", bufs=2, space="PSUM"))
        ps_z1p = ctx.enter_context(tc.tile_pool(name="psz1", bufs=1, space="PSUM"))
        ps_z2p = ctx.enter_context(tc.tile_pool(name="psz2", bufs=1, space="PSUM"))
        misc = ctx.enter_context(tc.tile_pool(name="misc", bufs=1))

        def cload(dram, shape, dtype, name):
            t = consts.tile(shape, dtype, tag=name)
            nc.sync.dma_start(t[:], dram[:])
            return t

        t_xq = cload(d_xq, [4, QPC], F32, "xq")
        t_xs = cload(d_xs, [4, N], F32, "xs")
        t_w10 = cload(d_w10, [1, HID], mdt, "w10")
        t_A = cload(d_A, [HID, HID], mdt, "A")
        t_Bm = cload(d_Bm, [HID, HID], mdt, "Bm")
        t_Cs = cload(d_Cs, [HID, HID], F32, "Cs")
        t_b10 = cload(d_b10, [HID, 1], F32, "b10")
        t_c1 = cload(d_c1, [HID, 1], F32, "c1")
        t_c2 = cload(d_c2, [HID, 1], F32, "c2")
        t_c3 = cload(d_c3, [HID, 1], F32, "c3")

        # ---- distances: d2 = xq^T xs (K=4), relu, sqrt ----
        ps_d = ps_a0p.tile([128, N], F32, tag="psa0")
        nc.tensor.matmul(ps_d[:], t_xq[:], t_xs[:], start=True, stop=True)
        d2_sb = misc.tile([128, N], F32, tag="d2")
        nc.scalar.activation(d2_sb[:], ps_d[:], AF.Relu)
        dist = misc.tile([128, N], mdt, tag="dist")
        nc.scalar.activation(dist[:], d2_sb[:], AF.Sqrt)

        # matmul operands must sit at base partition 0 — flatten dist rows
        # into partition-0 chunks via SBUF->SBUF DMA (CQ query rows each).
        CQ = 16
        dflat = ctx.enter_context(tc.tile_pool(name="dflat", bufs=2))

        # ---- per-query-pair fused MLP chain ----
        t_S = misc.tile([HID, QPC], F32, tag="S")
        W = N  # 512 free per query
        fl = None
        for p in range(QPC // QB):
            if (QB * p) % CQ == 0:
                c = (QB * p) // CQ
                fl = dflat.tile([1, CQ * W], mdt, tag="dflat")
                nc.sync.dma_start(fl[:], dist[CQ * c : CQ * (c + 1), :])
            ps_a0 = ps_a0p.tile([128, QB * W], F32, tag="psa0")
            for k in range(QB):
                q = (QB * p + k) % CQ
                nc.tensor.matmul(
                    ps_a0[:, k * W : (k + 1) * W],
                    _mm_ap(t_w10[:]),
                    _mm_ap(fl[0:1, q * W : (q + 1) * W]),
                    start=True,
                    stop=True,
                )
            a0 = sb.tile([128, QB * W], mdt, tag="a")
            nc.scalar.activation(a0[:], ps_a0[:], act, bias=t_b10[:])

            ps_z1 = ps_z1p.tile([128, QB * W], F32, tag="psz1")
            for k in range(QB):
                nc.tensor.matmul(
                    ps_z1[:, k * W : (k + 1) * W],
                    _mm_ap(t_A[:]),
                    _mm_ap(a0[:, k * W : (k + 1) * W]),
                    start=True,
                    stop=True,
                )
            a1 = sb.tile([128, QB * W], mdt, tag="a")
            nc.scalar.activation(a1[:], ps_z1[:], act, bias=t_c1[:])

            ps_z2 = ps_z2p.tile([128, QB * W], F32, tag="psz2")
            for k in range(QB):
                nc.tensor.matmul(
                    ps_z2[:, k * W : (k + 1) * W],
                    _mm_ap(t_Bm[:]),
                    _mm_ap(a1[:, k * W : (k + 1) * W]),
                    start=True,
                    stop=True,
                )
            a2 = sb.tile([128, QB * W], F32, tag="a2")
            nc.scalar.activation(a2[:], ps_z2[:], act, bias=t_c2[:])

            nc.vector.reduce_sum(
                t_S[:, QB * p : QB * (p + 1)],
                a2[:].rearrange("h (q j) -> h q j", q=QB),
                axis=mybir.AxisListType.X,
            )

        # ---- final projection: out[o, i] = sum_h Cs[h, o] S[h, i] + c3[o] ----
        ps_o = ps_z2p.tile([HID, QPC], F32, tag="psz2")
        nc.tensor.matmul(ps_o[:], t_Cs[:], t_S[:], start=True, stop=True)
        out_sb = misc.tile([HID, QPC], F32, tag="out")
        nc.scalar.activation(out_sb[:], ps_o[:], AF.Identity, bias=t_c3[:])
        nc.sync.dma_start(d_out[:], out_sb[:])

    nc.compile()
    return nc


def build_nc_sym():
    """Symmetric-pair sharding kernel (SPMD-uniform shapes).

    Per core (batch b = c//4, block k = c%4, node blocks I0..I3 of 128,
    self-block halves P = I_k[:64], Q = I_k[64:]):
      J3: 64 q x 128 src  ({k, k+2} half-pair)     rowsum + colsum
      JB: P x P and Q x Q (self-halves, full)      rowsum only
      JC: P x Q (cross half, computed once)        rowsum + colsum
      JA: I_k x I_{k+1}                            rowsum + colsum
    Device emits out_p = Cs^T @ S (128 o x 704 partial-query cols); the host
    gather adds partials into global query rows and adds c3 once.
    """
    nc = bacc.Bacc("TRN2", target_bir_lowering=False)
    mdt = _mm_tile_dt()

    d_xqA = nc.dram_tensor("xqA", [4, 128], F32, kind="ExternalInput")
    d_xsA = nc.dram_tensor("xsA", [4, 128], F32, kind="ExternalInput")
    d_xsD = nc.dram_tensor("xsD", [4, 128], F32, kind="ExternalInput")
    d_xqB = nc.dram_tensor("xqB", [4, 64], F32, kind="ExternalInput")
    d_xsB = nc.dram_tensor("xsB", [4, 128], F32, kind="ExternalInput")
    d_w10 = nc.dram_tensor("w10", [1, HID], mdt, kind="ExternalInput")
    d_A = nc.dram_tensor("A", [HID, HID], mdt, kind="ExternalInput")
    d_Bm = nc.dram_tensor("Bm", [HID, HID], mdt, kind="ExternalInput")
    d_Cs = nc.dram_tensor("Cs", [HID, HID], F32, kind="ExternalInput")
    d_b10 = nc.dram_tensor("b10", [HID, 1], F32, kind="ExternalInput")
    d_c1 = nc.dram_tensor("c1", [HID, 1], F32, kind="ExternalInput")
    d_c2 = nc.dram_tensor("c2", [HID, 1], F32, kind="ExternalInput")
    d_out = nc.dram_tensor("out", [HID, 704], F32, kind="ExternalOutput")

    act = _act_fn()

    with tile.TileContext(nc) as tc, ExitStack() as ctx:
        consts = ctx.enter_context(tc.tile_pool(name="consts", bufs=1))
        sb = ctx.enter_context(tc.tile_pool(name="sb", bufs=8))
        ps_a0p = ctx.enter_context(tc.tile_pool(name="psa0", bufs=2, space="PSUM"))
        ps_z1p = ctx.enter_context(tc.tile_pool(name="psz1", bufs=1, space="PSUM"))
        ps_z2p = ctx.enter_context(tc.tile_pool(name="psz2", bufs=1, space="PSUM"))
        misc = ctx.enter_context(tc.tile_pool(name="misc", bufs=1))
        ctmps = ctx.enter_context(tc.tile_pool(name="ctmps", bufs=3))
        dflat = ctx.enter_context(tc.tile_pool(name="dflat", bufs=3))

        _eng = [nc.sync, nc.gpsimd, nc.scalar]
        _ei = [0]

        def cload(dram, shape, dtype, name):
            t = consts.tile(shape, dtype, tag=name)
            _eng[_ei[0] % len(_eng)].dma_start(t[:], dram[:])
            _ei[0] += 1
            return t

        t_xqA = cload(d_xqA, [4, 128], F32, "xqA")
        t_xsA = cload(d_xsA, [4, 128], F32, "xsA")
        t_xsD = cload(d_xsD, [4, 128], F32, "xsD")
        t_xqB = cload(d_xqB, [4, 64], F32, "xqB")
        t_xsB = cload(d_xsB, [4, 128], F32, "xsB")
        t_w10 = cload(d_w10, [1, HID], mdt, "w10")
        t_A = cload(d_A, [HID, HID], mdt, "A")
        t_Bm = cload(d_Bm, [HID, HID], mdt, "Bm")
        t_Cs = cload(d_Cs, [HID, HID], F32, "Cs")
        t_b10 = cload(d_b10, [HID, 1], F32, "b10")
        t_c1 = cload(d_c1, [HID, 1], F32, "c1")
        t_c2 = cload(d_c2, [HID, 1], F32, "c2")

        def dist_block(qt, st, np_, nf, tagp):
            psd = ps_a0p.tile([np_, nf], F32, tag="psa0")
            nc.tensor.matmul(psd[:], qt[:], st[:], start=True, stop=True)
            d2 = misc.tile([np_, nf], F32, tag="d2" + tagp)
            nc.vector.tensor_scalar_max(d2[:], psd[:], 0.0)
            dd = misc.tile([np_, nf], mdt, tag="dist" + tagp)
            nc.scalar.activation(dd[:], d2[:], AF.Sqrt)
            return dd

        dist2 = dist_block(t_xqB, t_xsB, 64, 128, "b")   # J3 (64 q x 128)
        distd = dist_block(t_xqA, t_xsD, 128, 128, "d")  # self-block
        dist1 = dist_block(t_xqA, t_xsA, 128, 128, "a")  # JA (I_k x I_{k+1})

        t_S = misc.tile([HID, 704], F32, tag="S")
        nc.gpsimd.memset(t_S[:, 128:256], 0.0)
        nc.gpsimd.memset(t_S[:, 320:448], 0.0)
        nc.gpsimd.memset(t_S[:, 640:704], 0.0)

        def chain(ps_a0):
            fd = 1024
            a0 = sb.tile([128, fd], mdt, tag="a")
            nc.scalar.activation(a0[:], ps_a0[:], act, bias=t_b10[:])
            ps_z1 = ps_z1p.tile([128, fd], F32, tag="psz1")
            for k in range(0, fd, 512):
                nc.tensor.matmul(
                    ps_z1[:, k : k + 512], t_A[:], a0[:, k : k + 512],
                    start=True, stop=True,
                )
            a1 = sb.tile([128, fd], mdt, tag="a")
            nc.scalar.activation(a1[:], ps_z1[:], act, bias=t_c1[:])
            ps_z2 = ps_z2p.tile([128, fd], F32, tag="psz2")
            for k in range(0, fd, 512):
                nc.tensor.matmul(
                    ps_z2[:, k : k + 512], t_Bm[:], a1[:, k : k + 512],
                    start=True, stop=True,
                )
            a2 = sb.tile([128, fd], F32, tag="a2")
            nc.scalar.activation(a2[:], ps_z2[:], act, bias=t_c2[:])
            return a2

        def m0_group(fl, off):
            ps_a0 = ps_a0p.tile([128, 1024], F32, tag="psa0")
            for k in range(2):
                nc.tensor.matmul(
                    ps_a0[:, k * 512 : (k + 1) * 512],
                    t_w10[:],
                    fl[0:1, off + k * 512 : off + (k + 1) * 512],
                    start=True, stop=True,
                )
            return ps_a0

        def rowsum(a2, nq, scol):
            nc.vector.reduce_sum(
                t_S[:, scol : scol + nq],
                a2[:].rearrange("h (q j) -> h q j", q=nq),
                axis=mybir.AxisListType.X,
            )

        def colsum_add(a2, nq, nj, scol):
            ctmp = ctmps.tile([128, nj], F32, tag="ctmp")
            nc.vector.reduce_sum(
                ctmp[:],
                a2[:].rearrange("h (q j) -> h q j", q=nq).transpose([0, 2, 1]),
                axis=mybir.AxisListType.X,
            )
            nc.gpsimd.tensor_add(
                t_S[:, scol : scol + nj], t_S[:, scol : scol + nj], ctmp[:]
            )

        out_sb = misc.tile([HID, 704], F32, tag="out")

        def project(c0, c1_):
            ps_o = ps_z2p.tile([HID, c1_ - c0], F32, tag="psz2")
            nc.tensor.matmul(ps_o[:], t_Cs[:], t_S[:, c0:c1_], start=True,
                             stop=True)
            nc.scalar.copy(out_sb[:, c0:c1_], ps_o[:])
            nc.sync.dma_start(d_out[:, c0:c1_], out_sb[:, c0:c1_])

        # ---- J3: 8 groups of 8 queries x 128 sources ----
        fl3 = dflat.tile([1, 64 * 128], mdt, tag="dflat")
        nc.sync.dma_start(fl3[:], dist2[:, :])
        for p in range(8):
            a2 = chain(m0_group(fl3, p * 1024))
            rowsum(a2, 8, 256 + 8 * p)
            colsum_add(a2, 8, 128, 320)
        project(256, 448)

        # ---- JB: self-halves P x P and Q x Q (rowsum only) ----
        for half in range(2):
            flb = dflat.tile([1, 64 * 64], mdt, tag="dflat")
            nc.sync.dma_start(
                flb[:], distd[64 * half : 64 * (half + 1),
                              64 * half : 64 * (half + 1)]
            )
            for p in range(4):
                a2 = chain(m0_group(flb, p * 1024))
                rowsum(a2, 16, 448 + 64 * half + 16 * p)

        # ---- JC: P x Q cross half (rowsum + colsum) ----
        flc = dflat.tile([1, 64 * 64], mdt, tag="dflat")
        nc.sync.dma_start(flc[:], distd[0:64, 64:128])
        for p in range(4):
            a2 = chain(m0_group(flc, p * 1024))
            rowsum(a2, 16, 576 + 16 * p)
            colsum_add(a2, 16, 64, 640)
        project(448, 704)

        # ---- JA: I_k x I_{k+1}, 16 groups of 8 queries x 128 sources ----
        for c in range(4):
            fla = dflat.tile([1, 32 * 128], mdt, tag="dflat")
            nc.sync.dma_start(fla[:], dist1[32 * c : 32 * (c + 1), :])
            for pp in range(4):
                p = 4 * c + pp
                a2 = chain(m0_group(fla, pp * 1024))
                rowsum(a2, 8, 8 * p)
                colsum_add(a2, 8, 128, 128)
        project(0, 256)

    nc.compile()
    return nc


def _aug_q(x0, x1, nrm):
    return np.stack([x0, x1, nrm, np.ones_like(x0)]).astype(np.float32)


def _aug_s(x0, x1, nrm):
    return np.stack([-2.0 * x0, -2.0 * x1, np.ones_like(x0), nrm]).astype(np.float32)


def _sym_blocks(k):
    """Returns (Q3, S3) local-node index arrays for core block k."""
    I = [np.arange(128 * m, 128 * (m + 1)) for m in range(4)]
    if k < 2:
        return I[k + 2][:64], I[k]
    return I[k][64:], I[(k + 2) % 4]


def make_in_maps_sym(inputs):
    w = fold_weights(inputs)
    mdt_np = ml_dtypes.bfloat16 if MM_MODE == "bf16" else np.float32
    shared = {
        "w10": w["w10"].astype(mdt_np),
        "A": w["A"].astype(mdt_np),
        "Bm": w["Bm"].astype(mdt_np),
        "Cs": w["Cs"].astype(np.float32),
        "b10": w["b10"].astype(np.float32),
        "c1": w["c1"].reshape(HID, 1).astype(np.float32),
        "c2": w["c2"].reshape(HID, 1).astype(np.float32),
    }
    x = np.asarray(inputs["x"], np.float32)
    nrm = x[..., 0] ** 2 + x[..., 1] ** 2
    in_maps = []
    for c in range(N_CORES):
        b, k = c // 4, c % 4
        Ik = np.arange(128 * k, 128 * (k + 1))
        Inext = (Ik + 128) % 512
        Q3, S3 = _sym_blocks(k)
        x0, x1, nr = x[b, :, 0], x[b, :, 1], nrm[b]
        in_maps.append({
            "xqA": _aug_q(x0[Ik], x1[Ik], nr[Ik]),
            "xsA": _aug_s(x0[Inext], x1[Inext], nr[Inext]),
            "xsD": _aug_s(x0[Ik], x1[Ik], nr[Ik]),
            "xqB": _aug_q(x0[Q3], x1[Q3], nr[Q3]),
            "xsB": _aug_s(x0[S3], x1[S3], nr[S3]),
            **shared,
        })
    return in_maps


def combine_sym(outs, inputs):
    """outs: list of 8 per-core (128 o, 704) partial arrays.

    S columns: [0:128] JA-rowsum (I_k), [128:256] JA-colsum (I_{k+1}),
    [256:320] J3-rowsum (Q3), [320:448] J3-colsum (S3),
    [448:576] JB-rowsum (I_k), [576:640] JC-rowsum (I_k[:64]),
    [640:704] JC-colsum (I_k[64:]).
    """
    w = fold_weights(inputs)
    out = np.broadcast_to(
        w["c3"].astype(np.float32), (B, N, HID)
    ).copy()
    for c in range(N_CORES):
        b, k = c // 4, c % 4
        P = outs[c]
        Ik = np.arange(128 * k, 128 * (k + 1))
        Inext = (Ik + 128) % 512
        Q3, S3 = _sym_blocks(k)
        out[b, Ik, :] += P[:, 0:128].T + P[:, 448:576].T
        out[b, Inext, :] += P[:, 128:256].T
        out[b, Q3, :] += P[:, 256:320].T
        out[b, S3, :] += P[:, 320:448].T
        out[b, Ik[:64], :] += P[:, 576:640].T
        out[b, Ik[64:], :] += P[:, 640:704].T
    return out


def fold_weights(inputs):
    f64 = {k: np.asarray(v, np.float64) for k, v in inputs.items()}
    out = {}
    out["A"] = f64["W2_0"] @ f64["W1_1"]
    out["c1"] = f64["b2_0"] @ f64["W1_1"] + f64["b1_1"]
    out["Bm"] = f64["W2_1"] @ f64["W1_2"]
    out["c2"] = f64["b2_1"] @ f64["W1_2"] + f64["b1_2"]
    out["Cs"] = (f64["W2_2"] @ f64["Wo"]) / float(N)
    out["c3"] = f64["b2_2"] @ f64["Wo"] + f64["bo"]
    out["w10"] = f64["W1_0"].reshape(1, HID)
    out["b10"] = f64["b1_0"].reshape(HID, 1)
    return out


def make_in_maps(inputs):
    w = fold_weights(inputs)
    mdt_np = ml_dtypes.bfloat16 if MM_MODE == "bf16" else np.float32
    shared = {
        "w10": w["w10"].astype(mdt_np),
        "A": w["A"].astype(mdt_np),
        "Bm": w["Bm"].astype(mdt_np),
        "Cs": w["Cs"].astype(np.float32),
        "b10": w["b10"].astype(np.float32),
        "c1": w["c1"].reshape(HID, 1).astype(np.float32),
        "c2": w["c2"].reshape(HID, 1).astype(np.float32),
        "c3": w["c3"].reshape(HID, 1).astype(np.float32),
    }
    x = np.asarray(inputs["x"], np.float32)  # (B, N, 2)
    nrm = x[..., 0] ** 2 + x[..., 1] ** 2  # (B, N)
    in_maps = []
    for c in range(N_CORES):
        b = c // (N_CORES // B)
        i0 = (c % (N_CORES // B)) * QPC
        xq = np.stack(
            [
                x[b, i0 : i0 + QPC, 0],
                x[b, i0 : i0 + QPC, 1],
                nrm[b, i0 : i0 + QPC],
                np.ones(QPC, np.float32),
            ]
        ).astype(np.float32)
        xs = np.stack(
            [
                -2.0 * x[b, :, 0],
                -2.0 * x[b, :, 1],
                np.ones(N, np.float32),
                nrm[b, :],
            ]
        ).astype(np.float32)
        in_maps.append({"xq": xq, "xs": xs, **shared})
    return in_maps


_NC_CACHE = {}


def get_nc():
    key = (MM_MODE, ACT_NAME, QB, ALGO)
    if key not in _NC_CACHE:
        _NC_CACHE[key] = build_nc_sym() if ALGO == "sym" else build_nc()
    return _NC_CACHE[key]


def run(inputs, trace=False, tmpdir=None):
    """Run on 8 cores; returns (full_output, BassKernelResults)."""
    nc = get_nc()
    if ALGO == "sym":
        in_maps = make_in_maps_sym(inputs)
    else:
        in_maps = make_in_maps(inputs)
    try:
        res = run_bass_kernel_spmd(
            nc, in_maps, list(range(N_CORES)), trace=trace, tmpdir=tmpdir
        )
    except Exception:
        # transient NRT device errors (e.g. NRT_EXEC_UNIT_UNRECOVERABLE from a
        # prior wedged run) usually clear on retry
        res = run_bass_kernel_spmd(
            nc, in_maps, list(range(N_CORES)), trace=trace, tmpdir=tmpdir
        )
    if ALGO == "sym":
        return combine_sym([res.results[c]["out"] for c in range(N_CORES)],
                           inputs), res
    out = np.empty((B, N, HID), np.float32)
    for c in range(N_CORES):
        b = c // (N_CORES // B)
        i0 = (c % (N_CORES // B)) * QPC
        out[b, i0 : i0 + QPC, :] = res.results[c]["out"].T
    return out, res


def kernel(**inputs):
    out, _ = run(inputs)
    return out
